# revision 5
# baseline (speedup 1.0000x reference)
"""Trainium2 Bass kernel for nn_FFT: per-16-float-chunk 4x2 complex FFT.

The reference op reshapes x (B, 32, 256) -> (B, 16, 32, 4, 2, 2), treats the
trailing (4, 2, 2) as a 4x2 complex plane (last axis = re/im), applies a 2D FFT
over the (4, 2) plane, and writes real/imag concatenated:
out idx within each 16-float chunk = 4*k + c + 2*is_imag.

Because all twiddles of a 4x2 FFT are {±1, ±i}, the transform of each chunk of
16 consecutive floats is a fixed signed-add network: 3 butterfly stages,
48 real adds per chunk -> 10 strided tensor_tensor ops per SBUF region
(multi-dim access patterns process all chunks of a region per op).

Sharding: batch dim across 8 cores (2048 samples each); per core the data is a
flat contiguous block of 16,777,216 floats viewed as (128 partitions, 131072),
streamed through SBUF in 16 tiles of (128, 8192) with triple buffering.
Loads are issued on the SP HWDGE ring, stores on the ACT HWDGE ring; the rings
drain independently (~330 GB/s/core aggregate, ~92% of the per-core HBM cap).

Compute is split across two engines so it stays off the DMA critical path:
stages A+B are column-split between DVE (chunks [0,264), 1 elem/cyc/lane
@0.96 GHz) and Pool/GPSIMD (chunks [264,512), ~2.6 cyc/elem @1.2 GHz); DVE
then runs stage C for the whole tile, deferred by one tile so it overlaps the
next tile's Pool work (~17.3 us per engine per tile, under the ~25 us/tile DMA
cadence).  Pool gets only stages A/B on purpose: their access patterns have
unit-stride innermost runs, while stage C's (c,i)->(i2,c2) transpose needs
stride-2 innermost sources, which the GPSIMD Q7 ucode silently corrupts under
load (HW-only; CoreSim is clean) — observed as even-offset-only errors.
"""

import numpy as np

B, H, W = 16384, 32, 256
N_CORES = 8
PER_CORE = B // N_CORES                # 2048 samples
FLAT = PER_CORE * H * W                # 16,777,216 floats per core
P = 128
FREE = FLAT // P                       # 131072 floats per partition
F_TILE = 8192                          # floats per partition per tile
N_TILES = FREE // F_TILE               # 16
N_CHUNKS = F_TILE // 16                # 512 16-float chunks per partition-tile
DVE_CHUNKS = 264                       # DVE A/B chunk share (Pool gets the rest)

_CACHED_NC = None


def _stage_ab(eng, tin, tout, c0, c1):
    """Butterfly stages A+B on chunks [c0, c1): tin -> tout (A) -> tin (B).

    Chunk element index: idx = 4*r + 2*c + ri  (r=row 0..3, c=col 0..1, ri=re/im).
    All access patterns here have unit-stride innermost runs (len 2 or 4).
    """
    ti = tin[:, c0 * 16:c1 * 16]
    to = tout[:, c0 * 16:c1 * 16]
    i5 = ti.rearrange("p (n r c i) -> p n r c i", r=4, c=2, i=2)
    o5 = to.rearrange("p (n r c i) -> p n r c i", r=4, c=2, i=2)

    # Stage A (length-2 FFT over c):  s[r] = x[r,0]+x[r,1] -> (r, c=0, ri)
    #                                 d[r] = x[r,0]-x[r,1] -> (r, c=1, ri)
    eng.tensor_add(out=o5[:, :, :, 0, :], in0=i5[:, :, :, 0, :], in1=i5[:, :, :, 1, :])
    eng.tensor_sub(out=o5[:, :, :, 1, :], in0=i5[:, :, :, 0, :], in1=i5[:, :, :, 1, :])

    # Stage B (first level of FFT4 over r): y -> t where
    # t[2b] = y[b] + y[b+2], t[2b+1] = y[b] - y[b+2], b in {0,1};
    # m = 2*c + ri runs over the 4 trailing values of each row slot.
    sB = to.rearrange("p (n qh ql m) -> p n qh ql m", qh=2, ql=2, m=4)
    tB = ti.rearrange("p (n qh ql m) -> p n qh ql m", qh=2, ql=2, m=4)
    eng.tensor_add(out=tB[:, :, :, 0, :], in0=sB[:, :, 0, :, :], in1=sB[:, :, 1, :, :])
    eng.tensor_sub(out=tB[:, :, :, 1, :], in0=sB[:, :, 0, :, :], in1=sB[:, :, 1, :, :])


def _stage_c(eng, tin, tout):
    """Stage C (second level of FFT4, twiddle -i on the odd branch) on the
    full tile, writing the reference's output layout: out idx = 4*k + 2*ri + c.
    Sources have stride-2 innermost runs (the (c,i)->(i2,c2) transpose), so
    this stage must run on DVE (GPSIMD corrupts such patterns on HW)."""
    t5 = tin[:].rearrange("p (n q c i) -> p n q c i", q=4, c=2, i=2)
    f5 = tout[:].rearrange("p (n k i2 c2) -> p n k i2 c2", k=4, i2=2, c2=2)
    # F0 = t0 + t2 ; F2 = t0 - t2   (srcs transposed (c,i)->(i,c) to match dst order)
    t0 = t5[:, :, 0, :, :].transpose([0, 1, 3, 2])
    t2 = t5[:, :, 2, :, :].transpose([0, 1, 3, 2])
    eng.tensor_add(out=f5[:, :, 0, :, :], in0=t0, in1=t2)
    eng.tensor_sub(out=f5[:, :, 2, :, :], in0=t0, in1=t2)
    # F1 = t1 - i*t3 ; F3 = t1 + i*t3
    t1re = t5[:, :, 1, :, 0]
    t1im = t5[:, :, 1, :, 1]
    t3re = t5[:, :, 3, :, 0]
    t3im = t5[:, :, 3, :, 1]
    eng.tensor_add(out=f5[:, :, 1, 0, :], in0=t1re, in1=t3im)   # F1.re = t1.re + t3.im
    eng.tensor_sub(out=f5[:, :, 1, 1, :], in0=t1im, in1=t3re)   # F1.im = t1.im - t3.re
    eng.tensor_sub(out=f5[:, :, 3, 0, :], in0=t1re, in1=t3im)   # F3.re = t1.re - t3.im
    eng.tensor_add(out=f5[:, :, 3, 1, :], in0=t1im, in1=t3re)   # F3.im = t1.im + t3.re


def _emit_pipeline(nc, tc, pool, x, y, mybir):
    """One full pass over (P, FREE): load, butterfly, store per tile.

    Stage C + store of tile j are emitted during tile j+1 (software
    pipelining) so DVE's C work overlaps Pool's A/B work on the next tile.
    """
    H_TILE = F_TILE // 4

    def emit_c_and_store(prev):
        tin_p, tout_p, j_p = prev
        _stage_c(nc.vector, tin_p, tout_p)
        lo = j_p * F_TILE
        for s in range(4):
            nc.scalar.dma_start(
                out=y[:, lo + s * H_TILE:lo + (s + 1) * H_TILE],
                in_=tout_p[:, s * H_TILE:(s + 1) * H_TILE],
            )

    # GPSIMD warmup: a tiny scratch op so the Q7 tensor_tensor kernel's IRAM
    # load (~6us) and first-launch window happen before any real data work.
    warm = pool.tile([P, 64], mybir.dt.float32, tag="warm")
    nc.gpsimd.memset(warm[:, 32:], 0.0)
    nc.gpsimd.tensor_add(out=warm[:, :32], in0=warm[:, 32:], in1=warm[:, 32:])

    prev = None
    for j in range(N_TILES):
        lo = j * F_TILE
        tin = pool.tile([P, F_TILE], mybir.dt.float32, tag="tin")
        # Loads on the SP HWDGE ring, stores on the ACT HWDGE ring: the two
        # rings drain independently, nearly doubling streaming bandwidth vs a
        # single ring.  Four 1MB DMAs per direction per tile interleave best
        # with the compute cadence.
        for s in range(4):
            nc.sync.dma_start(
                out=tin[:, s * H_TILE:(s + 1) * H_TILE],
                in_=x[:, lo + s * H_TILE:lo + (s + 1) * H_TILE],
            )
        tout = pool.tile([P, F_TILE], mybir.dt.float32, tag="tout")
        if j == 0:
            # Tile 0 runs entirely on DVE: Pool's first real op would race
            # the still-arriving first loads on HW (observed corruption at
            # the start of load s=3's range, tile 0 only, timing-dependent
            # rows; CoreSim clean).  From tile 1 on, loads run ~2 tiles
            # ahead of compute, so the race window never reopens.
            _stage_ab(nc.vector, tin, tout, 0, N_CHUNKS)
        else:
            _stage_ab(nc.vector, tin, tout, 0, DVE_CHUNKS)
            _stage_ab(nc.gpsimd, tin, tout, DVE_CHUNKS, N_CHUNKS)
        if prev is not None:
            emit_c_and_store(prev)
        prev = (tin, tout, j)
    emit_c_and_store(prev)


def _build(reps: int = 1):
    from concourse import bacc
    import concourse.mybir as mybir
    from concourse.tile import TileContext

    nc = bacc.Bacc("TRN2", target_bir_lowering=False, debug=False)
    x = nc.dram_tensor("x", (P, FREE), mybir.dt.float32, kind="ExternalInput").ap()
    y = nc.dram_tensor("y", (P, FREE), mybir.dt.float32, kind="ExternalOutput").ap()

    with TileContext(nc) as tc:
        with tc.tile_pool(name="pool", bufs=3) as pool:
            for _ in range(reps):
                _emit_pipeline(nc, tc, pool, x, y, mybir)
    nc.compile()
    nc.finalize()
    return nc


def get_nc():
    global _CACHED_NC
    if _CACHED_NC is None:
        _CACHED_NC = _build()
    return _CACHED_NC


def kernel(x: np.ndarray, **_unused) -> np.ndarray:
    from concourse.bass_utils import run_bass_kernel_spmd

    x = np.ascontiguousarray(np.asarray(x, dtype=np.float32))
    assert x.shape == (B, H, W), x.shape
    nc = get_nc()
    xs = x.reshape(N_CORES, P, FREE)
    in_maps = [{"x": xs[i]} for i in range(N_CORES)]
    res = run_bass_kernel_spmd(nc, in_maps, core_ids=list(range(N_CORES)))
    out = np.stack([r["y"] for r in res.results])
    return out.reshape(B, H, W)
